# revision 14
# baseline (speedup 1.0000x reference)
"""Trainium2 Bass kernel for BaseSSMLayer (diagonal linear SSM).

Computation (verified equivalent to the reference's associative_scan):
    U = xs @ w_in.T              # [L, N]
    h_t = lam * h_{t-1} + U_t    # linear recurrence over L
    Y = H @ c_out.T + xs * d_skip

Strategy: tensor-parallel over state channels (N=2048 -> 256 per core,
8 cores, no cross-core communication).  Each core works in transposed
space (channels/out-dim on SBUF partitions, time on the free axis):

    matmul1 (TensorE, bf16): U_T[n, t] = w_sh.T @ xs_T      (contraction over in_dim)
    scan    (VectorE, f32 state): H_T[n, t] = lam*H_T[n, t-1] + U_T[n, t]
                             (hardware tensor_tensor_scan along free axis)
    matmul2 (TensorE, bf16): Yp_T[o, t] = c_sh.T @ H_T      (contraction over n-shard)

The 8 partial Yp_T are summed on the host (f32) and the diagonal skip
xs * d_skip is added there too.

Data layout: xs/y live in DRAM as [NT, 128, KI*TT] "slab" blocks so each
DMA moves 16 KiB-contiguous runs per partition (descriptor-rate limits
dominate at 1 KiB).  Host does the (cheap) permutations.
"""

import numpy as np
import ml_dtypes

import concourse.tile as tile
from concourse import bacc, mybir
from concourse.bass import ts
from concourse.bass_utils import run_bass_kernel_spmd

L = 16384        # sequence length
I = 2048         # in_dim (= out dim of Y)
N = 2048         # state_dim
NCORES = 8
NSH = N // NCORES        # 256 state channels per core
NG = NSH // 128          # 2 partition-groups of channels per core
TT = 512                 # time-tile (free dim per matmul / scan)
NT = L // TT             # 32 time tiles
KI = I // 128            # 16 contraction tiles over in_dim

BF16 = mybir.dt.bfloat16
F32 = mybir.dt.float32
NP_BF16 = ml_dtypes.bfloat16


def _build_nc():
    nc = bacc.Bacc(
        "TRN2",
        target_bir_lowering=False,
        debug=False,
        num_devices=NCORES,
    )
    xt = nc.dram_tensor("xt", [NT, 128, KI * TT], BF16, kind="ExternalInput").ap()
    wt = nc.dram_tensor("wt", [I, NSH], BF16, kind="ExternalInput").ap()
    ct = nc.dram_tensor("ct", [NSH, I], BF16, kind="ExternalInput").ap()
    lamb = nc.dram_tensor("lamb", [128, NG * TT], F32, kind="ExternalInput").ap()
    y = nc.dram_tensor("y", [NT, 128, KI * TT], BF16, kind="ExternalOutput").ap()

    NCH = 4           # DMA chunking: KI split into NCH chunks of KC i-tiles
    KC = KI // NCH

    with tile.TileContext(nc) as tc:
        with (
            tc.tile_pool(name="const", bufs=1) as const_pool,
            tc.tile_pool(name="xin", bufs=5 * NCH) as x_pool,
            tc.tile_pool(name="hb", bufs=10) as hb_pool,
            tc.tile_pool(name="yst", bufs=3 * NCH) as yst_pool,
            tc.tile_pool(name="ups", bufs=2, space="PSUM") as u_psum,
            tc.tile_pool(name="yps", bufs=6, space="PSUM") as y_psum,
        ):
            # --- resident constants (w chunked so the first MMs start early) ---
            w_sb = []
            for j in range(NCH):
                w = const_pool.tile([128, KC * NSH], BF16, tag=f"w{j}")
                nc.scalar.dma_start(
                    w[:].rearrange("p (i n) -> p i n", i=KC),
                    wt[j * KC * 128:(j + 1) * KC * 128, :].rearrange(
                        "(i p) n -> p i n", p=128
                    ),
                )
                w_sb.append(w)
            lam_sb = const_pool.tile([128, NG * TT], F32, tag="lam")
            nc.scalar.dma_start(lam_sb[:], lamb[:])
            c_sb = []
            for g in range(NG):
                c = const_pool.tile([128, I], BF16, tag=f"c{g}")
                nc.scalar.dma_start(c[:], ct[g * 128:(g + 1) * 128, :])
                c_sb.append(c)

            # Pre-warm the PE HAM clock gate during the initial DMA ramp so the
            # first real matmuls run at 2.4 GHz instead of 1.2 GHz.
            warm_sb = const_pool.tile([128, 128], BF16, tag="warm")
            nc.gpsimd.memset(warm_sb[:], 0.0)
            warm_ps = u_psum.tile([128, 128], F32, tag="u")
            for _ in range(40):
                nc.tensor.matmul(warm_ps[:], warm_sb[:], warm_sb[:], start=True, stop=True)

            hb_prev = [None] * NG          # bf16 H tile of previous slab, per group
            hb_tiles = [None] * (NT * NG)  # bf16 H tiles pending matmul2
            xt3 = xt.rearrange("t p (j c) -> t p j c", j=NCH)

            def emit_mm1(t):
                x_chunks = []
                for j in range(NCH):
                    xc = x_pool.tile([128, KC * TT], BF16, tag="x")
                    nc.sync.dma_start(xc[:], xt3[t, :, j])
                    x_chunks.append(xc)
                def mm1_one(u_ps, g, i):
                    j, ic = divmod(i, KC)
                    nc.tensor.matmul(
                        u_ps[:],
                        w_sb[j][:, ic * NSH + g * 128: ic * NSH + (g + 1) * 128],
                        x_chunks[j][:, ts(ic, TT)],
                        start=(i == 0),
                        stop=(i == KI - 1),
                    )

                u_list = []
                for g in range(NG):
                    u_ps = u_psum.tile([128, TT], F32, tag="u")
                    u_list.append(u_ps)
                if t == 0:
                    for i in range(KI):
                        for g in range(NG):
                            mm1_one(u_list[g], g, i)
                else:
                    for g in range(NG):
                        for i in range(KI):
                            mm1_one(u_list[g], g, i)
                for g in range(NG):
                    u_ps = u_list[g]
                    hb = hb_pool.tile([128, TT], BF16, tag="hb")
                    init = 0.0 if t == 0 else hb_prev[g][:, TT - 1: TT]
                    nc.vector.tensor_tensor_scan(
                        hb[:],
                        lam_sb[:, ts(g, TT)],
                        u_ps[:],
                        init,
                        op0=mybir.AluOpType.mult,
                        op1=mybir.AluOpType.add,
                    )
                    hb_prev[g] = hb
                    hb_tiles[t * NG + g] = hb

            def emit_mm2(t):
                for j in range(NCH):
                    y_stage = yst_pool.tile([128, KC * TT], BF16, tag="yst")
                    for oc in range(KC):
                        o = j * KC + oc
                        y_ps = y_psum.tile([128, TT], F32, tag="y")
                        for g in range(NG):
                            nc.tensor.matmul(
                                y_ps[:],
                                c_sb[g][:, ts(o, 128)],
                                hb_tiles[t * NG + g][:],
                                start=(g == 0),
                                stop=(g == NG - 1),
                            )
                        # drain PSUM -> bf16 staging; split across ACT and DVE
                        if oc in (0, KC - 1):
                            nc.vector.tensor_copy(y_stage[:, ts(oc, TT)], y_ps[:])
                        else:
                            nc.scalar.copy(y_stage[:, ts(oc, TT)], y_ps[:])
                    nc.scalar.dma_start(yo3[t, :, j], y_stage[:])

            yo3 = y.rearrange("t p (j c) -> t p j c", j=NCH)

            # software-pipelined: matmul2 for slab t runs one slab behind
            # matmul1, so the PE never waits on the scan chain.
            for t in range(NT + 1):
                if t < NT:
                    emit_mm1(t)
                if t >= 1:
                    emit_mm2(t - 1)

    nc.compile()
    return nc


_NC_CACHE = None


def _get_nc():
    global _NC_CACHE
    if _NC_CACHE is None:
        _NC_CACHE = _build_nc()
    return _NC_CACHE


def _prep_in_maps(xs, lam, w_in, c_out):
    # xs.T -> [KI, 128, NT, TT] -> [NT, 128, KI, TT] slabs (16 KiB runs/partition)
    xt = (
        np.ascontiguousarray(xs.T)
        .astype(NP_BF16)
        .reshape(KI, 128, NT, TT)
        .transpose(2, 1, 0, 3)
        .reshape(NT, 128, KI * TT)
    )
    xt = np.ascontiguousarray(xt)
    w_t = np.ascontiguousarray(w_in.T)                        # [I, N]
    c_t = np.ascontiguousarray(c_out.T)                       # [N, I]
    in_maps = []
    for k in range(NCORES):
        sh = slice(k * NSH, (k + 1) * NSH)
        wt = np.ascontiguousarray(w_t[:, sh]).astype(NP_BF16)     # [I, NSH]
        ct = np.ascontiguousarray(c_t[sh, :]).astype(NP_BF16)     # [NSH, I]
        lam_sh = lam[sh].reshape(NG, 128).astype(np.float32)      # [g, p]
        lamb = np.ascontiguousarray(
            np.broadcast_to(lam_sh[:, :, None], (NG, 128, TT))
            .transpose(1, 0, 2)
            .reshape(128, NG * TT)
        )
        in_maps.append({"xt": xt, "wt": wt, "ct": ct, "lamb": lamb})
    return in_maps


def combine_outputs(results, xs, d_skip):
    """results: list of per-core {"y": [NT, 128, KI*TT] bf16} -> full Y [L, I] f32."""
    acc = results[0]["y"].astype(np.float32)
    for r in results[1:]:
        acc += r["y"].astype(np.float32)
    # [NT, 128, KI, TT] -> Y_T [I, L] -> Y [L, I]
    y_t = acc.reshape(NT, 128, KI, TT).transpose(2, 1, 0, 3).reshape(I, L)
    out = y_t.T + xs * d_skip[None, :].astype(np.float32)
    return np.ascontiguousarray(out, dtype=np.float32)


def run_on_hw(xs, lam, w_in, c_out, d_skip):
    """Returns (Y full f32 [L, I], BassKernelResults)."""
    nc = _get_nc()
    in_maps = _prep_in_maps(xs, lam, w_in, c_out)
    res = run_bass_kernel_spmd(nc, in_maps, core_ids=list(range(NCORES)))
    return combine_outputs(res.results, xs, d_skip), res


def kernel(xs, lam, w_in, c_out, d_skip):
    out, _ = run_on_hw(
        np.asarray(xs, dtype=np.float32),
        np.asarray(lam, dtype=np.float32),
        np.asarray(w_in, dtype=np.float32),
        np.asarray(c_out, dtype=np.float32),
        np.asarray(d_skip, dtype=np.float32),
    )
    return out
